# revision 16
# baseline (speedup 1.0000x reference)
"""Sliding-window attention (w=256) on 8 TRN2 NeuronCores.

Problem: q,k,v [b=2, s=4096, h=8, d=64] fp32, each query attends keys within
+/-256. Sharding: b*h = 16 head-slices; each core takes 2 ADJACENT heads of one
batch so every DMA row is 512B-contiguous (full line rate).

Per-core algorithm (heads h0,h1; 16 chunks of 256 queries):
  - Load q,k per 512-col batch (HWDGE fp32), PE-transpose the fp32 tiles
    directly (2 cyc/row), DVE-copy psum->SBUF with the bf16 cast folded in.
    qT2/kT2 [128(=2*64 d-stacked), 512]-tiles: partitions 0:64 = head0's d,
    64:128 = head1's d.
  - v cast to bf16 per head via gpsimd SWDGE: v_ext [128, 36, 65] (s-tiles on
    partitions, padded 2 tiles each side, 65th column of ones -> softmax
    denominator falls out of the ctx matmul for free).
  - Scores TRANSPOSED: S^T[y, x] = k.q (K=d=64), bf16, PACKED psum layout
    [128, 1280] = [j1 | j2 | j3 | j4 | j0 x-lo | j5 x-hi] - quarter tiles no
    ctx matmul consumes are never computed nor exp'd.
  - One exp per chunk-head on ACT with the 1/sqrt(d) scale folded in; band
    masking via [128,128] triangle-mask multiplies (DVE+Pool) after exp.
  - ctx[x, 65] = sum_j E_j[:, x-half].T @ v_ext[2c+j] (bf16) into ONE psum
    bank [128, 4, 65]; 1 reciprocal + 4 scalar-muls per chunk; out DMAs
    grouped (5 per body).
  - The chunk loop is software-pipelined: scores(c+1, h) are emitted BEFORE
    ctx(c, h) so PE fills the next psum slot while ACT drains exp(c); DMA
    issue runs ~3 chunks ahead of the transposes that consume the staged
    tiles, which in turn run ~2 chunks ahead of the scores that need them.
  - The benchmark For_i loop holds 4 kernel bodies per iteration so the
    inter-iteration all-engine barrier amortizes; bodies overlap through the
    rotating tile pools.
"""

import numpy as np

import concourse.bass as bass
import concourse.bacc as bacc
import concourse.mybir as mybir
from concourse.tile import TileContext
from concourse.bass_utils import run_bass_kernel_spmd
from concourse.masks import make_identity

F32 = mybir.dt.float32
BF16 = mybir.dt.bfloat16

S = 4096
D = 64
W = 256
C = S // W  # 16 chunks
NT = S // 128  # 32 s-tiles
NB = NT // 4  # 8 transpose batches of 4 tiles

# packed E/psum layout, PSUM-bank aligned (bank = 512 fp32): bank0 = [j1|j2],
# bank1 = [j3|j4], half bank2 = [j0 x-lo | j5 x-hi]; every scores matmul's
# output stays inside one bank.
EW = 1280

# out-DMA groups: chunks per group (last kept solo for tail latency)
OGROUPS = [(0, 4), (4, 4), (8, 4), (12, 3), (15, 1)]


def _eoff(j, xt):
    if j == 0:
        assert xt == 0
        return 1024
    if j == 5:
        assert xt == 1
        return 1152
    return 256 * (j - 1) + 128 * xt


_CACHE = {}


def build_nc(repeats=1, loop_n=0):
    nc = bacc.Bacc("TRN2", target_bir_lowering=False)
    q = nc.dram_tensor("q", [S, 128], F32, kind="ExternalInput")
    k = nc.dram_tensor("k", [S, 128], F32, kind="ExternalInput")
    v = nc.dram_tensor("v", [S, 128], F32, kind="ExternalInput")
    out = nc.dram_tensor("out", [S, 128], F32, kind="ExternalOutput")

    with TileContext(nc) as tc:
        with (
            tc.tile_pool(name="const", bufs=1) as constp,
            tc.tile_pool(name="big", bufs=1) as bigp,
            tc.tile_pool(name="stage", bufs=8) as stagep,
            tc.tile_pool(name="spsum", bufs=2, space="PSUM") as spsum,
            tc.tile_pool(name="xpsum", bufs=2, space="PSUM") as xpsum,
            tc.tile_pool(name="epool", bufs=4) as epool,
            tc.tile_pool(name="rpool", bufs=4) as rpool,
            tc.tile_pool(name="opool", bufs=3) as opool,
        ):
            # ---- constants ----
            identf = constp.tile([128, 128], F32, name="identf")
            make_identity(nc, identf)
            # triangle masks [128, 128]: tle keeps x <= p, tge keeps x >= p
            tle = constp.tile([128, 128], BF16, name="tle")
            tge = constp.tile([128, 128], BF16, name="tge")
            for t, cm in ((tle, 1), (tge, -1)):
                nc.gpsimd.memset(t, 1.0)
                nc.gpsimd.affine_select(
                    out=t, in_=t,
                    compare_op=mybir.AluOpType.is_ge,
                    fill=0.0, base=0,
                    pattern=[[-cm, 128]],
                    channel_multiplier=cm,
                )

            # warm the ACT exp table set during phase A (hides ~2.7us load)
            warm = constp.tile([128, 1], F32, name="warm")
            nc.vector.memset(warm, 0.0)
            nc.scalar.activation(warm, warm, mybir.ActivationFunctionType.Exp)

            # ---- persistent buffers ----
            BATCHES = [(4 * b, 4) for b in range(NB)]
            qT = [bigp.tile([128, 128 * n], BF16, name=f"qT{b}")
                  for b, (_, n) in enumerate(BATCHES)]
            kT = [bigp.tile([128, 128 * n], BF16, name=f"kT{b}")
                  for b, (_, n) in enumerate(BATCHES)]
            TSTART = [128 * t0 for t0, _ in BATCHES]
            vext = [bigp.tile([128, NT, D + 1], BF16, name=f"vext{h}") for h in range(2)]

            def _bat(off):
                for b in range(len(BATCHES) - 1, -1, -1):
                    if TSTART[b] <= off:
                        return b, off - TSTART[b]
                raise AssertionError(off)

            def kslice(g):
                """kT2 view at padded-global col g, width 128 (in-range only)."""
                assert W <= g < W + S
                b, off = _bat(g - W)
                return kT[b][:, off:off + 128]

            def qslice(x0, w):
                b, off = _bat(x0)
                return qT[b][:, off:off + w]

            qr = q[:, :].rearrange("(t p) f -> p t f", p=128)
            kr = k[:, :].rearrange("(t p) f -> p t f", p=128)
            vr = v[:, :].rearrange("(t p) (h d) -> p t h d", p=128, h=2)

            def emit_all():
                stf = {}  # (which, b) -> staged fp32 tile

                def emit_dma(which, b, halves=1, eng=None, only_half=None):
                    t0, n = BATCHES[b]
                    srcr = qr if which == "q" else kr
                    if (which, b) in stf:
                        st = stf[(which, b)]
                    else:
                        st = stagep.tile([128, 4, 128], F32, name="stf", tag="stf")
                        stf[(which, b)] = st
                    eng = eng or nc.sync
                    m = n // halves
                    for hh in range(halves):
                        if only_half is not None and hh != only_half:
                            continue
                        sl = slice(hh * m, hh * m + m)
                        eng.dma_start(st[:, sl, :],
                                      srcr[:, t0 + hh * m:t0 + hh * m + m, :])

                def emit_tp(which, b):
                    """Transpose staged fp32 tiles into bf16 qT/kT (the
                    fp32->bf16 cast happens on the PE output path)."""
                    st = stf.pop((which, b))
                    dst = (qT if which == "q" else kT)[b]
                    n = BATCHES[b][1]
                    tp = xpsum.tile([128, 512], F32, name="tp", tag="x")
                    for i in range(n):
                        nc.tensor.transpose(tp[:, 128 * i:128 * (i + 1)],
                                            st[:, i, :], identf)
                    nc.vector.tensor_copy(dst[:, 0:128 * n], tp[:, 0:128 * n])

                def emit_tp_half(which, b, hh):
                    """Half-batch transpose+copy for the startup batches."""
                    st = stf[(which, b)]
                    dst = (qT if which == "q" else kT)[b]
                    tp = (xpsum.tile([128, 512], F32, name="tph", tag="x")
                          if hh == 0 else _tp_half[(which, b)])
                    _tp_half[(which, b)] = tp
                    for i in range(2 * hh, 2 * hh + 2):
                        nc.tensor.transpose(tp[:, 128 * i:128 * (i + 1)],
                                            st[:, i, :], identf)
                    nc.vector.tensor_copy(dst[:, 256 * hh:256 * hh + 256],
                                          tp[:, 256 * hh:256 * hh + 256])
                    if hh == 1:
                        stf.pop((which, b))
                        _tp_half.pop((which, b))

                _tp_half = {}

                sp = {}  # (c, h) -> scores psum tile

                def emit_scores(c, h):
                    jlo = [0, 1]
                    jhi = [4, 5]
                    if c == 0:
                        jlo = [2, 2]
                    if c == C - 1:
                        jhi = [3, 3]
                    t = spsum.tile([128, EW], F32, name=f"sp{h}", tag="sp")
                    sp[(c, h)] = t
                    jrange = list(range(min(jlo), max(jhi) + 1))
                    if c == 0:
                        jrange = [0] + jrange
                    for j in jrange:
                        if j == 0:
                            xs, xw = 0, 128
                        elif j == 5:
                            xs, xw = 128, 128
                        else:
                            xs, xw = 0, 256
                        eo = _eoff(j, 1 if j == 5 else 0)
                        nc.tensor.matmul(
                            t[:, eo:eo + xw],
                            lhsT=kslice(max(W * c + 128 * j, W))[64 * h:64 * h + 64, :],
                            rhs=qslice(W * c + xs, xw)[64 * h:64 * h + 64, :],
                            start=True, stop=True,
                            tile_position=(64 * h, 0),
                        )

                E = {}

                def emit_exp(c, h):
                    if c == 0:
                        espans = [(256, 1280)]
                    elif c == C - 1:
                        espans = [(0, 768), (1024, 1152)]
                    else:
                        espans = [(0, 1280)]
                    t = sp.pop((c, h))
                    e = epool.tile([128, EW], BF16, name="E")
                    E[(c, h)] = e
                    for e0, e1 in espans:
                        nc.scalar.activation(e[:, e0:e1], t[:, e0:e1],
                                             mybir.ActivationFunctionType.Exp,
                                             scale=float(D) ** -0.5)
                    # band masks on the consumed partial tiles (E *= 0/1)
                    if c != 0:
                        nc.gpsimd.tensor_tensor(e[:, 1024:1152], e[:, 1024:1152], tle,
                                                mybir.AluOpType.mult)
                        nc.vector.tensor_tensor(e[:, 128:256], e[:, 128:256], tle,
                                                mybir.AluOpType.mult)
                    if c != C - 1:
                        nc.vector.tensor_tensor(e[:, 768:896], e[:, 768:896], tge,
                                                mybir.AluOpType.mult)
                        nc.gpsimd.tensor_tensor(e[:, 1152:1280], e[:, 1152:1280], tge,
                                                mybir.AluOpType.mult)

                xc = {}

                def emit_ctx(c, h):
                    jlo = [0, 1]
                    jhi = [4, 5]
                    if c == 0:
                        jlo = [2, 2]
                    if c == C - 1:
                        jhi = [3, 3]
                    if h == 0:
                        xc[c] = xpsum.tile([128, 4, D + 1], F32, name="xc", tag="x")
                    e = E[(c, h)] if h == 0 else E.pop((c, h))
                    if h == 1:
                        E.pop((c, 0))
                    for xt in range(2):
                        ctx = xc[c][:, 2 * h + xt, :]
                        js = list(range(jlo[xt], jhi[xt] + 1))
                        for j in js:
                            eo = _eoff(j, xt)
                            nc.tensor.matmul(
                                ctx,
                                lhsT=e[:, eo:eo + 128],
                                rhs=vext[h][:, 2 * c + j - 2, :],
                                start=(j == js[0]), stop=(j == js[-1]),
                            )

                ost = {}  # group index -> ostage tile

                def emit_norm(c, h):
                    gi, (g0, glen) = next(
                        (i, g) for i, g in enumerate(OGROUPS)
                        if g[0] <= c < g[0] + g[1])
                    if c == g0 and h == 0:
                        ost[gi] = opool.tile([128, 2 * glen, 128], F32, name="ost")
                    t = xc[c] if h == 0 else xc.pop(c)
                    rc = rpool.tile([128, 2, 1], F32, name="rc")
                    nc.vector.reciprocal(rc, t[:, 2 * h:2 * h + 2, D:D + 1])
                    for xt in range(2):
                        nc.vector.tensor_scalar_mul(
                            ost[gi][:, 2 * (c - g0) + xt, 64 * h:64 * h + 64],
                            t[:, 2 * h + xt, 0:D],
                            rc[:, xt, :])
                    if h == 0:
                        return
                    outr = out[:, :].rearrange("(n p) f -> p n f", p=128)
                    if gi < 3 and c == g0 + glen - 1:
                        nc.sync.dma_start(
                            outr[:, 2 * g0:2 * g0 + 2 * glen, :], ost.pop(gi))
                    elif gi == 4:
                        # last chunk via the idle ACT queue, then the deferred
                        # g3 group on SP (both data-ready: no queue blocking)
                        nc.scalar.dma_start(outr[:, 2 * g0:2 * g0 + 2, :],
                                            ost.pop(gi))
                        og0, oglen = OGROUPS[3]
                        nc.sync.dma_start(
                            outr[:, 2 * og0:2 * og0 + 2 * oglen, :], ost.pop(3))

                # ---- startup: prioritize exactly what scores(0) needs:
                # k tiles 0-5 and q tiles 0-1, split across the SP and ACT
                # HWDGE queues so issue overlaps; everything else follows.
                emit_dma("k", 0)                                        # SP
                emit_dma("k", 1, halves=2, eng=nc.scalar, only_half=0)  # ACT
                emit_dma("q", 0, halves=2, only_half=0)                 # SP
                emit_dma("k", 1, halves=2, eng=nc.scalar, only_half=1)  # ACT
                emit_dma("q", 0, halves=2, only_half=1)                 # SP
                # PE p-state warmup: tiny junk matmuls into the first two sp
                # ring slots keep PE continuously busy through the DMA-load
                # window so the first real transposes/scores run at full clock
                # (ramp needs ~3us of uninterrupted execution).
                for _ in range(2):
                    junk = spsum.tile([128, EW], F32, name="spw", tag="sp")
                    for i in range(16):
                        nc.tensor.matmul(junk[:, 0:16], lhsT=identf,
                                         rhs=identf[:, 0:16],
                                         start=True, stop=True)
                emit_tp("k", 0)
                emit_tp_half("k", 1, 0)
                emit_tp_half("q", 0, 0)
                # v: 2 SWDGE cast-DMAs per head (cast + head-split in DGE)
                for h in range(2):
                    nc.vector.memset(vext[h][:, :, D:D + 1], 1.0)
                for h in range(2):
                    nc.gpsimd.dma_start(vext[h][:, 0:16, 0:D], vr[:, 0:16, h, :])
                emit_dma("q", 1)                        # SP

                emit_scores(0, 0)
                emit_tp_half("k", 1, 1)
                emit_tp_half("q", 0, 1)
                emit_scores(0, 1)

                dma_b = 2
                tp_k = 2
                tp_q = 1
                dma_chunk = {1: -2}

                def feed_batches(c):
                    nonlocal dma_b, tp_k, tp_q
                    if c == 2:
                        # v second halves (not read before chunk 7)
                        for h in range(2):
                            nc.gpsimd.dma_start(vext[h][:, 16:32, 0:D],
                                                vr[:, 16:32, h, :])
                    while dma_b < NB and 4 * dma_b <= 2 * (c + 3) + 5:
                        emit_dma("k", dma_b)
                        emit_dma("q", dma_b)
                        dma_chunk[dma_b] = c
                        dma_b += 1
                    # stagger k/q transposes into different chunks to bound
                    # per-chunk PE work
                    if tp_k < dma_b and dma_chunk[tp_k] <= c - 2:
                        emit_tp("k", tp_k)
                        tp_k += 1
                    if tp_q < tp_k and dma_chunk[tp_q] <= c - 3:
                        emit_tp("q", tp_q)
                        tp_q += 1

                for c in range(C):
                    emit_exp(c, 0)
                    emit_exp(c, 1)
                    feed_batches(c)
                    if c + 1 < C:
                        emit_scores(c + 1, 0)
                    emit_ctx(c, 0)
                    emit_norm(c, 0)
                    if c + 1 < C:
                        emit_scores(c + 1, 1)
                    emit_ctx(c, 1)
                    emit_norm(c, 1)

            if loop_n:
                import os
                n_unroll = int(os.environ.get("KERN_UNROLL", "4"))
                if loop_n % n_unroll or loop_n < n_unroll:
                    n_unroll = 1
                with tc.For_i(0, loop_n // n_unroll, 1):
                    for _ in range(n_unroll):
                        emit_all()
            else:
                for _ in range(repeats):
                    emit_all()
    nc.compile()
    return nc


def kernel(q, k, v, w):
    q = np.asarray(q, dtype=np.float32)
    k = np.asarray(k, dtype=np.float32)
    v = np.asarray(v, dtype=np.float32)
    assert int(w) == W
    if "nc" not in _CACHE:
        _CACHE["nc"] = build_nc()
    nc = _CACHE["nc"]
    in_maps = []
    for core in range(8):
        b = core // 4
        h0 = 2 * (core % 4)
        in_maps.append({
            "q": np.ascontiguousarray(q[b, :, h0:h0 + 2, :]).reshape(S, 128),
            "k": np.ascontiguousarray(k[b, :, h0:h0 + 2, :]).reshape(S, 128),
            "v": np.ascontiguousarray(v[b, :, h0:h0 + 2, :]).reshape(S, 128),
        })
    res = run_bass_kernel_spmd(nc, in_maps, core_ids=list(range(8)))
    out = np.empty((2, S, 8, D), np.float32)
    for core, om in enumerate(res.results):
        b = core // 4
        h0 = 2 * (core % 4)
        out[b, :, h0:h0 + 2, :] = om["out"].reshape(S, 2, D)
    return out
